# revision 1
# baseline (speedup 1.0000x reference)
"""Deformable CNN block (offset conv -> deformable conv -> sync-BN -> ReLU)
as a Bass/Tile kernel for 8 Trainium2 NeuronCores, data-parallel over batch.

Per core (one batch item):
  - conv grid ("m-grid"): 114x114 padded grid, m=(ho+1)*114+(wo+1), padded
    to N=13312 = 13 groups x 1024. Packed plane layout: group g lives at
    partitions 32*(g//4)+k (k=tap), free block (g%4)*1024.
  - sample grid ("s-grid"): 118x118 zero-ringed image (image origin (2,2));
    a DRAM token table holds, per slot j, the channel vectors of the four
    bilinear corners (j, j+1, j+118, j+119) = 256 bf16 = 512B.
  - per tap: dma_gather(transpose=True, single_packet=False) pulls
    [128=(64c|64c), 2, n] corner tiles, selector matmuls replicate compact
    bilinear coefs across partitions into PSUM, DVE forms the coef-weighted
    rhs, PE accumulates the deformable conv over taps/corners in PSUM.
  - sync-BN: per-channel sum/sumsq, AllReduce over 8 cores, fused
    scale+shift+ReLU on the scalar engine.
"""

import numpy as np
import ml_dtypes

import concourse.bass as bass
import concourse.bacc as bacc
import concourse.mybir as mybir
from concourse.bass_utils import run_bass_kernel_spmd
from concourse.tile import TileContext

F32 = mybir.dt.float32
BF16 = mybir.dt.bfloat16
I16 = mybir.dt.int16
BF = ml_dtypes.bfloat16

H = W = 112
C_IN, C_OUT, KK = 64, 128, 9
CG = 114
SG = 118
N = 13312
NG = CG * CG          # 12996
NGROUP, GSTR = 13, 1024
PBLK = 32
TOK = SG * SG         # 13924
TOKPAD = 13952
XC_OFF = 115
XCN = N + 2 * XC_OFF + 4
CHUNK = 512
NCHUNK = N // CHUNK
GSLICE = 1024
EPS = 1e-5

ADD = mybir.AluOpType.add
MULT = mybir.AluOpType.mult
SUB = mybir.AluOpType.subtract
MAXOP = mybir.AluOpType.max
MINOP = mybir.AluOpType.min


def _base_planes():
    m = np.arange(N)
    ry = (m // CG).astype(np.float32)
    rx = (m % CG).astype(np.float32)
    bY = np.zeros((128, 4 * GSTR), np.float32)
    bX = np.zeros((128, 4 * GSTR), np.float32)
    for g in range(NGROUP):
        sl = slice(g * GSTR, (g + 1) * GSTR)
        fb = slice((g % 4) * GSTR, (g % 4 + 1) * GSTR)
        for k in range(KK):
            p = PBLK * (g // 4) + k
            bY[p, fb] = ry[sl] + (k // 3)
            bX[p, fb] = rx[sl] + (k % 3)
    return bY, bX


def build_nc(w_off, b_off, w_dcn, b_dcn, gamma, beta, hw_loop=1, n_cores=8,
             no_cc=False, no_gather=False):
    nc = bacc.Bacc("TRN2", target_bir_lowering=False, num_devices=n_cores)

    x_in = nc.dram_tensor("x", [C_IN, H, W], F32, kind="ExternalInput")
    y_out = nc.dram_tensor("y", [C_OUT, H, W], F32, kind="ExternalOutput")

    # ---- host-prepacked constants ----
    w_off_r = w_off.reshape(KK, 2, C_IN, 3, 3)
    w_perm = np.concatenate([w_off_r[:, 0], w_off_r[:, 1]], 0)      # [18,64,3,3]
    b_perm = np.concatenate(
        [b_off.reshape(KK, 2)[:, 0], b_off.reshape(KK, 2)[:, 1]])   # [18]
    woff18 = np.stack(
        [w_perm[:, :, ky, kx].T for ky in range(3) for kx in range(3)], 1)
    woff_taps = np.zeros((C_IN, KK, 41), np.float32)
    woff_taps[:, :, 0:9] = woff18[:, :, 0:9]
    woff_taps[:, :, 32:41] = woff18[:, :, 9:18]
    woffT_c = nc.inline_tensor(woff_taps.astype(BF), name="woffT")
    bY128 = np.zeros((128, 1), np.float32)
    bX128 = np.zeros((128, 1), np.float32)
    for b in range(4):
        bY128[PBLK * b:PBLK * b + 9, 0] = b_perm[0:9]
        bX128[PBLK * b:PBLK * b + 9, 0] = b_perm[9:18]
    boffY_c = nc.inline_tensor(bY128, name="boffY")
    boffX_c = nc.inline_tensor(bX128, name="boffX")
    bY_np, bX_np = _base_planes()
    bY_c = nc.inline_tensor(bY_np.astype(BF), name="baseY")
    bX_c = nc.inline_tensor(bX_np.astype(BF), name="baseX")
    wd = w_dcn.reshape(C_OUT, C_IN, 3, 3)
    wdup = np.stack(
        [np.concatenate([wd[:, :, k // 3, k % 3].T] * 2, 0) for k in range(KK)], 1)
    wdup_c = nc.inline_tensor(wdup.astype(BF), name="wdup")         # [128,9,128]
    bdcn_c = nc.inline_tensor(b_dcn.reshape(C_OUT, 1).astype(np.float32), name="bdcn")
    gam_c = nc.inline_tensor(gamma.reshape(C_OUT, 1).astype(np.float32), name="gam")
    bet_c = nc.inline_tensor(beta.reshape(C_OUT, 1).astype(np.float32), name="bet")
    id64_c = nc.inline_tensor(np.eye(64, dtype=BF), name="id64")
    # selector: lhsT slice j=(g//4)*9+k picks packed row 32*(g//4)+k
    sel_np = np.zeros((128, 36, 64), np.float32)
    for b in range(4):
        for k in range(KK):
            sel_np[PBLK * b + k, b * 9 + k, :] = 1.0
    sel_c = nc.inline_tensor(sel_np.astype(BF), name="sel3")

    tok_dram = nc.dram_tensor("tok", [TOKPAD, 256], BF16)
    xc_dram = nc.dram_tensor("xcd", [C_IN, XCN], BF16)
    gidx_dram = nc.dram_tensor("gidxd", [KK, N], I16)
    stats_in = nc.dram_tensor("statin", [C_OUT, 2], F32)
    stats_out = nc.dram_tensor("statout", [C_OUT, 2], F32, addr_space="Shared")

    with TileContext(nc) as tc:
        with (
            tc.tile_pool(name="big", bufs=1) as big,
            tc.tile_pool(name="work", bufs=2) as work,
            tc.tile_pool(name="ps", bufs=1, space="PSUM") as pp,
            tc.tile_pool(name="psy", bufs=2, space="PSUM") as ppy,
        ):
            loop = tc.For_i(0, hw_loop, 1) if hw_loop > 1 else None
            if loop is not None:
                loop.__enter__()

            # ---- stage 1+2: bf16 grids; token table + xc -> DRAM ----
            with tc.tile_pool(name="sc12", bufs=1) as sc12:
                xs = sc12.tile([64, TOK + 256], BF16, tag="xs")
                nc.vector.memset(xs[:], 0.0)
                xs_img = xs[:, 0:TOK].rearrange(
                    "p (r c) -> p r c", r=SG)[:, 2:114, 2:114]
                nc.gpsimd.dma_start(out=xs_img, in_=x_in[:])
                xcs = sc12.tile([64, XCN], BF16, tag="xcs")
                nc.vector.memset(xcs[:], 0.0)
                xc_img = xcs[:, XC_OFF:XC_OFF + NG].rearrange(
                    "p (r c) -> p r c", r=CG)[:, 1:113, 1:113]
                nc.gpsimd.dma_start(out=xc_img, in_=x_in[:])
                nc.sync.dma_start(out=xc_dram[:], in_=xcs[:])
                idt = sc12.tile([64, 64], BF16, tag="idt")
                nc.sync.dma_start(out=idt[:], in_=id64_c[:])
                for t in range((TOK + 127) // 128):  # 109
                    j0 = t * 128
                    pst = pp.tile([128, 256], BF16, tag="pst")
                    for di, dlt in enumerate((0, 1, SG, SG + 1)):
                        nc.tensor.transpose(
                            out=pst[:, di * 64:(di + 1) * 64],
                            in_=xs[:, j0 + dlt:j0 + dlt + 128],
                            identity=idt[:],
                        )
                    stg = work.tile([128, 256], BF16, tag="stg")
                    nc.vector.tensor_copy(stg[:], pst[:])
                    nc.sync.dma_start(out=tok_dram[j0:j0 + 128, :], in_=stg[:])

            # ---- stages 3-5 ----
            cTL = big.tile([128, 4 * GSTR], BF16, tag="cTL")
            cTR = big.tile([128, 4 * GSTR], BF16, tag="cTR")
            cBL = big.tile([128, 4 * GSTR], BF16, tag="cBL")
            cBR = big.tile([128, 4 * GSTR], BF16, tag="cBR")
            gidx_w = big.tile([128, KK, N // 16], I16, tag="gidx_w")
            with tc.tile_pool(name="sc34", bufs=1) as sc34:
                pyP = sc34.tile([128, 4 * GSTR], F32, tag="pyP")
                pxP = sc34.tile([128, 4 * GSTR], F32, tag="pxP")
                nc.vector.memset(pyP[:], 0.0)
                nc.vector.memset(pxP[:], 0.0)
                scr = sc34.tile([128, 4 * GSTR], F32, tag="scr")
                woT = sc34.tile([64, KK, 41], BF16, tag="woT")
                boY = sc34.tile([128, 1], F32, tag="boY")
                boX = sc34.tile([128, 1], F32, tag="boX")
                bY = sc34.tile([128, 4 * GSTR], BF16, tag="bY")
                bX = sc34.tile([128, 4 * GSTR], BF16, tag="bX")
                nc.sync.dma_start(out=woT[:], in_=woffT_c[:])
                nc.sync.dma_start(out=boY[:], in_=boffY_c[:])
                nc.sync.dma_start(out=boX[:], in_=boffX_c[:])
                nc.sync.dma_start(out=bY[:], in_=bY_c[:])
                nc.sync.dma_start(out=bX[:], in_=bX_c[:])
                for ch in range(NCHUNK):
                    n0 = ch * CHUNK
                    xcc = work.tile([64, CHUNK + 2 * XC_OFF], BF16, tag="xcc")
                    nc.sync.dma_start(
                        out=xcc[:], in_=xc_dram[:, n0:n0 + CHUNK + 2 * XC_OFF])
                    psc = pp.tile([41, CHUNK], F32, tag="psc")
                    for k in range(KK):
                        dlt = (k // 3 - 1) * CG + (k % 3 - 1)
                        nc.tensor.matmul(
                            psc[:], woT[:, k, :],
                            xcc[:, XC_OFF + dlt:XC_OFF + dlt + CHUNK],
                            start=(k == 0), stop=(k == KK - 1),
                        )
                    g, hf = ch // 2, ch % 2
                    rows = slice(PBLK * (g // 4), PBLK * (g // 4) + 9)
                    cols = slice((g % 4) * GSTR + hf * CHUNK,
                                 (g % 4) * GSTR + (hf + 1) * CHUNK)
                    nc.vector.scalar_tensor_tensor(
                        out=pyP[rows, cols], in0=psc[0:9, :], scalar=boY[rows],
                        in1=bY[rows, cols], op0=ADD, op1=ADD)
                    nc.vector.scalar_tensor_tensor(
                        out=pxP[rows, cols], in0=psc[32:41, :], scalar=boX[rows],
                        in1=bX[rows, cols], op0=ADD, op1=ADD)

                # indices + bilinear coefs (bY/bX freed into wy1/wx1 roles)
                y0b = sc34.tile([128, 4 * GSTR], BF16, tag="y0b")
                x0b = sc34.tile([128, 4 * GSTR], BF16, tag="x0b")
                wy1 = bY
                wx1 = bX
                nc.vector.tensor_scalar(scr[:], pyP[:], 8388607.5, None, ADD)
                nc.vector.tensor_scalar(y0b[:], scr[:], -8388608.0, None, ADD)
                nc.vector.tensor_tensor(wy1[:], pyP[:], y0b[:], SUB)
                nc.vector.tensor_scalar(scr[:], pxP[:], 8388607.5, None, ADD)
                nc.vector.tensor_scalar(x0b[:], scr[:], -8388608.0, None, ADD)
                nc.vector.tensor_tensor(wx1[:], pxP[:], x0b[:], SUB)
                nc.vector.tensor_scalar(y0b[:], y0b[:], 0.0, 115.0, MAXOP, MINOP)
                nc.vector.tensor_scalar(x0b[:], x0b[:], 0.0, 115.0, MAXOP, MINOP)
                gidx_f = pyP
                nc.vector.scalar_tensor_tensor(
                    out=gidx_f[:], in0=y0b[:], scalar=float(SG), in1=x0b[:],
                    op0=MULT, op1=ADD)
                gidx_i = sc34.tile([128, 4 * GSTR], I16, tag="gidx_i")
                nc.vector.tensor_copy(gidx_i[:], gidx_f[:])
                wy0 = y0b
                wx0 = x0b
                nc.vector.tensor_scalar(wy0[:], wy1[:], -1.0, 1.0, MULT, ADD)
                nc.vector.tensor_scalar(wx0[:], wx1[:], -1.0, 1.0, MULT, ADD)
                nc.vector.tensor_tensor(cTL[:], wy0[:], wx0[:], MULT)
                nc.vector.tensor_tensor(cTR[:], wy0[:], wx1[:], MULT)
                nc.vector.tensor_tensor(cBL[:], wy1[:], wx0[:], MULT)
                nc.vector.tensor_tensor(cBR[:], wy1[:], wx1[:], MULT)

                for g in range(NGROUP):
                    nc.sync.dma_start(
                        out=bass.AP(gidx_dram, g * GSTR, [[N, KK], [1, GSTR]]),
                        in_=gidx_i[PBLK * (g // 4):PBLK * (g // 4) + 9,
                                   (g % 4) * GSTR:(g % 4 + 1) * GSTR],
                    )
                for k in range(KK):
                    srcap = bass.AP(gidx_dram, k * N, [[1, 16], [16, N // 16]])
                    for grp in range(8):
                        nc.sync.dma_start(
                            out=gidx_w[grp * 16:(grp + 1) * 16, k, :], in_=srcap)

            # ---- stage 6: deformable conv main loop ----
            ydef = big.tile([C_OUT, N], BF16, tag="ydef")
            bd = big.tile([C_OUT, 1], F32, tag="bd")
            nc.sync.dma_start(out=bd[:], in_=bdcn_c[:])
            wdupS = big.tile([128, KK, 128], BF16, tag="wdupS")
            nc.sync.dma_start(out=wdupS[:], in_=wdup_c[:])
            sel3 = big.tile([128, 36, 64], BF16, tag="sel3")
            nc.sync.dma_start(out=sel3[:], in_=sel_c[:])

            for s in range(NGROUP):
                psyA = ppy.tile([C_OUT, CHUNK], F32, tag="psyA")
                psyB = ppy.tile([C_OUT, CHUNK], F32, tag="psyB")
                for k in range(KK):
                    gt = work.tile([128, 2, GSLICE], BF16, tag="gt")
                    if no_gather:
                        nc.vector.memset(gt[:], 0.0)
                    else:
                        nc.gpsimd.dma_gather(
                            out_ap=gt[:], in_ap=tok_dram[:],
                            idxs_ap=gidx_w[:, k, s * (GSLICE // 16):(s + 1) * (GSLICE // 16)],
                            num_idxs=GSLICE, num_idxs_reg=GSLICE, elem_size=256,
                            transpose=True, single_packet=False,
                        )
                    for cc in range(GSLICE // CHUNK):  # 2
                        selk = sel3[:, (s // 4) * 9 + k, :]
                        cols = slice((s % 4) * GSTR + cc * CHUNK,
                                     (s % 4) * GSTR + (cc + 1) * CHUNK)
                        ctT = pp.tile([128, CHUNK], F32, tag="ctT")
                        ctB = pp.tile([128, CHUNK], F32, tag="ctB")
                        nc.tensor.matmul(ctT[0:64, :], selk,
                                         cTL[:, cols], start=True, stop=True)
                        nc.tensor.matmul(ctT[64:128, :], selk,
                                         cTR[:, cols], start=True, stop=True)
                        nc.tensor.matmul(ctB[0:64, :], selk,
                                         cBL[:, cols], start=True, stop=True)
                        nc.tensor.matmul(ctB[64:128, :], selk,
                                         cBR[:, cols], start=True, stop=True)
                        gT = work.tile([128, CHUNK], BF16, tag="gT")
                        gB = work.tile([128, CHUNK], BF16, tag="gB")
                        nc.vector.tensor_tensor(
                            gT[:], gt[:, 0, cc * CHUNK:(cc + 1) * CHUNK], ctT[:], MULT)
                        nc.vector.tensor_tensor(
                            gB[:], gt[:, 1, cc * CHUNK:(cc + 1) * CHUNK], ctB[:], MULT)
                        psy = psyA if cc == 0 else psyB
                        nc.tensor.matmul(psy[:], wdupS[:, k, :], gT[:],
                                         start=(k == 0), stop=False,
                                         skip_group_check=True)
                        nc.tensor.matmul(psy[:], wdupS[:, k, :], gB[:],
                                         start=False, stop=(k == KK - 1),
                                         skip_group_check=True)
                for cc in range(GSLICE // CHUNK):
                    n0 = s * GSLICE + cc * CHUNK
                    psy = psyA if cc == 0 else psyB
                    nc.vector.tensor_scalar(
                        ydef[:, n0:n0 + CHUNK], psy[:], bd[:], None, ADD)

            # ---- stage 7: BN stats + AllReduce ----
            ssum = big.tile([C_OUT, 8], F32, tag="ssum")
            ssq = big.tile([C_OUT, 8], F32, tag="ssq")
            yv = ydef[:, 0:NG].rearrange("p (r c) -> p r c", r=CG)
            for r in range(7):
                vap = yv[:, 1 + r * 16:1 + (r + 1) * 16, 1:113]
                nc.vector.tensor_reduce(
                    ssum[:, r:r + 1], vap, axis=mybir.AxisListType.XY, op=ADD)
                sqscr = work.tile([C_OUT, 16 * W], F32, tag="ofin")
                nc.vector.scalar_tensor_tensor(
                    out=sqscr[:].rearrange("p (a b) -> p a b", a=16), in0=vap,
                    scalar=1.0, in1=vap, op0=MULT, op1=MULT,
                    accum_out=ssq[:, r:r + 1])
            st2 = big.tile([C_OUT, 2], F32, tag="st2")
            nc.vector.tensor_reduce(
                st2[:, 0:1], ssum[:, 0:7], axis=mybir.AxisListType.X, op=ADD)
            nc.vector.tensor_reduce(
                st2[:, 1:2], ssq[:, 0:7], axis=mybir.AxisListType.X, op=ADD)
            nc.sync.dma_start(out=stats_in[:], in_=st2[:])
            if no_cc:
                nc.sync.dma_start(out=stats_out[:], in_=stats_in[:])
            else:
                nc.gpsimd.collective_compute(
                    "AllReduce", ADD, replica_groups=[list(range(n_cores))],
                    ins=[stats_in[:]], outs=[stats_out[:]])
            stg2 = big.tile([C_OUT, 2], F32, tag="stg2")
            nc.sync.dma_start(out=stg2[:], in_=stats_out[:])

            # ---- stage 8: BN affine + ReLU + store ----
            gam = big.tile([C_OUT, 1], F32, tag="gamt")
            bet = big.tile([C_OUT, 1], F32, tag="bett")
            nc.sync.dma_start(out=gam[:], in_=gam_c[:])
            nc.sync.dma_start(out=bet[:], in_=bet_c[:])
            NTOT = float(n_cores * H * W)
            mean = big.tile([C_OUT, 1], F32, tag="mean")
            var = big.tile([C_OUT, 1], F32, tag="var")
            nc.vector.tensor_scalar(mean[:], stg2[:, 0:1], 1.0 / NTOT, None, MULT)
            nc.vector.tensor_scalar(var[:], stg2[:, 1:2], 1.0 / NTOT, None, MULT)
            m2 = big.tile([C_OUT, 1], F32, tag="m2")
            nc.vector.tensor_tensor(m2[:], mean[:], mean[:], MULT)
            nc.vector.tensor_tensor(var[:], var[:], m2[:], SUB)
            nc.vector.tensor_scalar(var[:], var[:], EPS, None, ADD)
            sd = big.tile([C_OUT, 1], F32, tag="sd")
            nc.scalar.activation(sd[:], var[:], mybir.ActivationFunctionType.Sqrt)
            rsd = big.tile([C_OUT, 1], F32, tag="rsd")
            nc.vector.reciprocal(rsd[:], sd[:])
            aa = big.tile([C_OUT, 1], F32, tag="aa")
            bb2 = big.tile([C_OUT, 1], F32, tag="bb2")
            nc.vector.tensor_tensor(aa[:], gam[:], rsd[:], MULT)
            nc.vector.tensor_tensor(bb2[:], aa[:], mean[:], MULT)
            nc.vector.tensor_tensor(bb2[:], bet[:], bb2[:], SUB)
            for r in range(7):
                vap = yv[:, 1 + r * 16:1 + (r + 1) * 16, 1:113]
                ofin = work.tile([C_OUT, 16 * W], F32, tag="ofin")
                nc.scalar.activation(
                    ofin[:].rearrange("p (a b) -> p a b", a=16), vap,
                    mybir.ActivationFunctionType.Relu, bias=bb2[:], scale=aa[:])
                nc.sync.dma_start(
                    out=y_out[:, r * 16:(r + 1) * 16, :],
                    in_=ofin[:].rearrange("p (a b) -> p a b", a=16))

            if loop is not None:
                loop.__exit__(None, None, None)

    nc.compile()
    return nc


def kernel(x, w_off, b_off, w_dcn, b_dcn, gamma, beta):
    x = np.asarray(x, np.float32)
    nc = build_nc(
        np.asarray(w_off, np.float32), np.asarray(b_off, np.float32),
        np.asarray(w_dcn, np.float32), np.asarray(b_dcn, np.float32),
        np.asarray(gamma, np.float32), np.asarray(beta, np.float32),
    )
    in_maps = [{"x": np.ascontiguousarray(x[b])} for b in range(8)]
    res = run_bass_kernel_spmd(nc, in_maps, list(range(8)))
    return np.stack([res.results[b]["y"] for b in range(8)], 0).astype(np.float32)



# revision 11
# speedup vs baseline: 3.2877x; 3.2877x over previous
"""Deformable CNN block (offset conv -> deformable conv -> sync-BN -> ReLU)
as a Bass/Tile kernel for 8 Trainium2 NeuronCores, data-parallel over batch.

Per core (one batch item):
  - s-grid image xs [64ch, y*128+x] bf16 (2-wide zero ring); DRAM token
    table row v = y0*128+x0 holds the 4 bilinear corners [TL TR BL BR]
    (256 bf16 = 512B), built by 4 XBAR dma_start_transpose slabs.
  - offset conv reads the c-grid image straight from SBUF; per group the
    positions/coefs/token-ids are computed in small per-group tiles.
  - coef planes cT/cB [128, 4096] bf16 carry TL/BL at partitions 32b+k
    and TR/BR at 32b+16+k, so one selector matmul per corner-pair
    replicates both across the 128 output rows.
  - per (group, tap): dma_gather pulls [128,2,1024] corner tiles from the
    DRAM token table, ACT copies the PSUM coef tile to bf16, DVE applies
    the bilinear weights at 2 elem/lane, PE accumulates the conv in PSUM.
  - sync-BN: windowed sum/sumsq interleaved with the group loop,
    AllReduce over 8 cores, fused scale+shift+ReLU on ACT.
"""

import numpy as np
import ml_dtypes

import concourse.bass as bass
import concourse.bacc as bacc
import concourse.mybir as mybir
from concourse.tile import TileContext

F32 = mybir.dt.float32
BF16 = mybir.dt.bfloat16
I16 = mybir.dt.int16
BF = ml_dtypes.bfloat16

H = W = 112
C_IN, C_OUT, KK = 64, 128, 9
CG = 114
N = 13312
NG = CG * CG            # 12996
NGROUP, GSTR = 13, 1024
PBLK = 32
SG = 118
XST = 128               # x-stride of padded s-grid image
TOKF = SG * XST         # 15104 = transpose window
XS_F = TOKF + 256
XC_OFF = 115
XCN = N + 2 * XC_OFF + 4
CHUNK = 512
NCHUNK = N // CHUNK     # 26
EPS = 1e-5

ADD = mybir.AluOpType.add
MULT = mybir.AluOpType.mult
SUB = mybir.AluOpType.subtract
MAXOP = mybir.AluOpType.max
MINOP = mybir.AluOpType.min


def _base_planes():
    m = np.arange(N)
    ry = (m // CG).astype(np.float32)
    rx = (m % CG).astype(np.float32)
    bY = np.zeros((128, 4 * GSTR), np.float32)
    bX = np.zeros((128, 4 * GSTR), np.float32)
    for g in range(NGROUP):
        sl = slice(g * GSTR, (g + 1) * GSTR)
        fb = slice((g % 4) * GSTR, (g % 4 + 1) * GSTR)
        for k in range(KK):
            p = PBLK * (g // 4) + k
            bY[p, fb] = ry[sl] + (k // 3)
            bX[p, fb] = rx[sl] + (k % 3)
    return bY, bX


def build_nc(w_off, b_off, w_dcn, b_dcn, gamma, beta, hw_loop=1, n_cores=8,
             no_cc=False, no_gather=False, const_idx=False):
    nc = bacc.Bacc("TRN2", target_bir_lowering=False, num_devices=n_cores)

    x_in = nc.dram_tensor("x", [C_IN, H, W], F32, kind="ExternalInput")
    y_out = nc.dram_tensor("y", [C_OUT, H, W], F32, kind="ExternalOutput")

    # ---- host-prepacked constants ----
    w_off_r = w_off.reshape(KK, 2, C_IN, 3, 3)
    w_perm = np.concatenate([w_off_r[:, 0], w_off_r[:, 1]], 0)      # [18,64,3,3]
    b_perm = np.concatenate(
        [b_off.reshape(KK, 2)[:, 0], b_off.reshape(KK, 2)[:, 1]])   # [18]
    woff18 = np.stack(
        [w_perm[:, :, ky, kx].T for ky in range(3) for kx in range(3)], 1)
    woff_taps = np.zeros((C_IN, KK, 41), np.float32)
    woff_taps[:, :, 0:9] = woff18[:, :, 0:9]
    woff_taps[:, :, 32:41] = woff18[:, :, 9:18]
    woffT_c = nc.inline_tensor(woff_taps.astype(BF), name="woffT")
    bY128 = np.zeros((128, 1), np.float32)
    bX128 = np.zeros((128, 1), np.float32)
    for b in range(4):
        bY128[PBLK * b:PBLK * b + 9, 0] = b_perm[0:9]
        bX128[PBLK * b:PBLK * b + 9, 0] = b_perm[9:18]
    boffY_c = nc.inline_tensor(bY128, name="boffY")
    boffX_c = nc.inline_tensor(bX128, name="boffX")
    bY_np, bX_np = _base_planes()
    bY_c = nc.inline_tensor(bY_np.astype(BF), name="baseY")
    bX_c = nc.inline_tensor(bX_np.astype(BF), name="baseX")
    wd = w_dcn.reshape(C_OUT, C_IN, 3, 3)
    wdup = np.stack(
        [np.concatenate([wd[:, :, k // 3, k % 3].T] * 2, 0) for k in range(KK)], 1)
    wdup_c = nc.inline_tensor(wdup.astype(BF), name="wdup")         # [128,9,128]
    bdcn_c = nc.inline_tensor(b_dcn.reshape(C_OUT, 1).astype(np.float32), name="bdcn")
    gam_c = nc.inline_tensor(gamma.reshape(C_OUT, 1).astype(np.float32), name="gam")
    bet_c = nc.inline_tensor(beta.reshape(C_OUT, 1).astype(np.float32), name="bet")
    # selector: slice j=(g//4)*9+k maps row 32b+k -> out 0..63 (TL/BL) and
    # row 32b+16+k -> out 64..127 (TR/BR)
    sel_np = np.zeros((128, 36, 128), np.float32)
    for b in range(4):
        for k in range(KK):
            sel_np[PBLK * b + k, b * 9 + k, 0:64] = 1.0
            sel_np[PBLK * b + 16 + k, b * 9 + k, 64:128] = 1.0
    sel_c = nc.inline_tensor(sel_np.astype(BF), name="sel2")
    cidx_c = nc.inline_tensor(np.full((128, 64), 1000, np.int16), name="cidx")

    gidx_dram = nc.dram_tensor("gidxd", [NGROUP, KK, GSTR], I16)
    tok_dram = nc.dram_tensor("tokd", [TOKF, 256], BF16)
    stats_in = nc.dram_tensor("statin", [C_OUT, 2], F32)
    stats_out = nc.dram_tensor("statout", [C_OUT, 2], F32, addr_space="Shared")

    with TileContext(nc) as tc:
        with (
            tc.tile_pool(name="big", bufs=1) as big,
            tc.tile_pool(name="work", bufs=2) as work,
            tc.tile_pool(name="gscr", bufs=2) as gscr,
            tc.tile_pool(name="wk3", bufs=3) as wk3,
        ):
            loop = tc.For_i(0, hw_loop, 1) if hw_loop > 1 else None
            if loop is not None:
                loop.__enter__()

            # ---- stage 1+2: image -> 4 transpose slabs -> DRAM token table
            with tc.tile_pool(name="s12", bufs=2) as s12p, \
                 tc.tile_pool(name="s12x", bufs=1) as s12x:
                xs = s12x.tile([64, XS_F], BF16, tag="xs")
                nc.gpsimd.memset(xs[:, 0:2 * XST], 0.0)
                nc.gpsimd.memset(xs[:, 114 * XST:XS_F], 0.0)
                xs_rows = xs[:, 0:TOKF].rearrange("p (r c) -> p r c", c=XST)
                nc.gpsimd.memset(xs_rows[:, 2:114, 114:128], 0.0)
                nc.gpsimd.memset(xs_rows[:, 2:114, 0:2], 0.0)
                nc.gpsimd.dma_start(
                    out=xs_rows[:, 2:114, 2:114], in_=x_in[:])
                for di, dlt in enumerate((0, 1, XST, XST + 1)):
                    slab = s12p.tile([128, SG, 64], BF16, tag="slab")
                    nc.sync.dma_start_transpose(
                        out=slab[:], in_=xs[:, dlt:dlt + TOKF])
                    nc.sync.dma_start(
                        out=bass.AP(tok_dram, di * 64,
                                    [[256, 128], [XST * 256, SG], [1, 64]]),
                        in_=slab[:])

            # ---- fused stages 3-6, per group ----
            main_cm = tc.tile_pool(name="main", bufs=1)
            mp = main_cm.__enter__()
            cT = mp.tile([128, 4 * GSTR], BF16, tag="cT")
            cB = mp.tile([128, 4 * GSTR], BF16, tag="cB")
            nc.gpsimd.memset(cT[:], 0.0)
            nc.gpsimd.memset(cB[:], 0.0)
            gidx_w = mp.tile([128, NGROUP, KK, 64], I16, tag="gidx_w")
            bY = mp.tile([128, 4 * GSTR], BF16, tag="bY")
            bX = mp.tile([128, 4 * GSTR], BF16, tag="bX")
            nc.sync.dma_start(out=bY[:], in_=bY_c[:])
            nc.sync.dma_start(out=bX[:], in_=bX_c[:])
            xcs = mp.tile([64, XCN], BF16, tag="xcs")
            xc_rows = xcs[:, XC_OFF:XC_OFF + NG].rearrange(
                "p (r c) -> p r c", c=CG)
            nc.gpsimd.memset(xcs[:, 0:XC_OFF + CG], 0.0)
            nc.gpsimd.memset(xc_rows[:, 1:113, 0:1], 0.0)
            nc.gpsimd.memset(xc_rows[:, 1:113, 113:114], 0.0)
            nc.gpsimd.memset(xcs[:, XC_OFF + 113 * CG:XCN], 0.0)
            xc_img = xcs[:, XC_OFF:XC_OFF + NG].rearrange(
                "p (r c) -> p r c", r=CG)[:, 1:113, 1:113]
            nc.gpsimd.dma_start(out=xc_img, in_=x_in[:])
            woT = mp.tile([64, KK, 41], BF16, tag="woT")
            boY = mp.tile([128, 1], F32, tag="boY")
            boX = mp.tile([128, 1], F32, tag="boX")
            nc.sync.dma_start(out=woT[:], in_=woffT_c[:])
            nc.sync.dma_start(out=boY[:], in_=boffY_c[:])
            nc.sync.dma_start(out=boX[:], in_=boffX_c[:])
            ydef = mp.tile([C_OUT, N], BF16, tag="ydef")
            bd = mp.tile([C_OUT, 1], F32, tag="bd")
            nc.sync.dma_start(out=bd[:], in_=bdcn_c[:])
            wdupS = mp.tile([128, KK, 128], BF16, tag="wdupS")
            nc.sync.dma_start(out=wdupS[:], in_=wdup_c[:])
            sel2 = mp.tile([128, 36, 128], BF16, tag="sel2")
            nc.sync.dma_start(out=sel2[:], in_=sel_c[:])
            cidx = mp.tile([128, 64], I16, tag="cidx")
            nc.sync.dma_start(out=cidx[:], in_=cidx_c[:])
            ssum = big.tile([C_OUT, 8], F32, tag="ssum")
            ssq = big.tile([C_OUT, 8], F32, tag="ssq")
            yv = ydef[:, 0:NG].rearrange("p (r c) -> p r c", r=CG)

            ps3cm = tc.tile_pool(name="ps3", bufs=2, space="PSUM")
            ps3 = ps3cm.__enter__()
            ps6cm = tc.tile_pool(name="ps6", bufs=2, space="PSUM")
            ps6 = ps6cm.__enter__()
            ppycm = tc.tile_pool(name="psy", bufs=1, space="PSUM")
            ppy = ppycm.__enter__()
            s78cm = tc.tile_pool(name="s78", bufs=2)
            s78 = s78cm.__enter__()

            bn_next = [0]

            def bn_windows(done_cols):
                while bn_next[0] < 7:
                    r = bn_next[0]
                    if (16 * r + 17) * CG > done_cols:
                        break
                    vap = yv[:, 1 + r * 16:1 + (r + 1) * 16, 1:113]
                    nc.vector.tensor_reduce(
                        ssum[:, r:r + 1], vap,
                        axis=mybir.AxisListType.XY, op=ADD)
                    sqscr = s78.tile([C_OUT, 16 * W], F32, tag="sqscr")
                    nc.vector.scalar_tensor_tensor(
                        out=sqscr[:].rearrange("p (a b) -> p a b", a=16),
                        in0=vap, scalar=1.0, in1=vap, op0=MULT, op1=MULT,
                        accum_out=ssq[:, r:r + 1])
                    bn_next[0] += 1

            for g in range(NGROUP):
                b = g // 4
                cb0 = (g % 4) * GSTR
                r32 = slice(PBLK * b, PBLK * b + 9)
                # offset conv for this group's two 512-chunks
                pyg = gscr.tile([16, GSTR], F32, tag="pyg")
                pxg = gscr.tile([16, GSTR], F32, tag="pxg")
                for hf in range(2):
                    n0 = g * GSTR + hf * CHUNK
                    psc = ps3.tile([41, CHUNK], F32, tag="psc")
                    for k in range(KK):
                        dlt = (k // 3 - 1) * CG + (k % 3 - 1)
                        nc.tensor.matmul(
                            psc[:], woT[:, k, :],
                            xcs[:, XC_OFF + n0 + dlt:XC_OFF + n0 + dlt + CHUNK],
                            start=(k == 0), stop=(k == KK - 1),
                        )
                    cols = slice(cb0 + hf * CHUNK, cb0 + (hf + 1) * CHUNK)
                    hcol = slice(hf * CHUNK, (hf + 1) * CHUNK)
                    nc.vector.scalar_tensor_tensor(
                        out=pyg[0:9, hcol], in0=psc[0:9, :], scalar=boY[r32],
                        in1=bY[r32, cols], op0=ADD, op1=ADD)
                    nc.vector.scalar_tensor_tensor(
                        out=pxg[0:9, hcol], in0=psc[32:41, :], scalar=boX[r32],
                        in1=bX[r32, cols], op0=ADD, op1=ADD)
                # pointwise for the group
                y0g = gscr.tile([16, GSTR], BF16, tag="y0g")
                x0g = gscr.tile([16, GSTR], BF16, tag="x0g")
                wy1g = gscr.tile([16, GSTR], BF16, tag="wy1g")
                wx1g = gscr.tile([16, GSTR], BF16, tag="wx1g")
                nc.vector.tensor_scalar(
                    y0g[0:9], pyg[0:9], 8388607.5, -8388608.0, ADD, ADD)
                nc.vector.tensor_tensor(wy1g[0:9], pyg[0:9], y0g[0:9], SUB)
                nc.vector.tensor_scalar(y0g[0:9], y0g[0:9], 0.0, 115.0,
                                        MAXOP, MINOP)
                nc.vector.tensor_scalar(
                    x0g[0:9], pxg[0:9], 8388607.5, -8388608.0, ADD, ADD)
                nc.vector.tensor_tensor(wx1g[0:9], pxg[0:9], x0g[0:9], SUB)
                nc.vector.tensor_scalar(x0g[0:9], x0g[0:9], 0.0, 115.0,
                                        MAXOP, MINOP)
                nc.vector.scalar_tensor_tensor(
                    out=pxg[0:9], in0=y0g[0:9], scalar=float(XST),
                    in1=x0g[0:9], op0=MULT, op1=ADD)
                gsm = work.tile([16, GSTR], I16, tag="gsm")
                nc.vector.tensor_copy(gsm[0:9], pxg[0:9])
                nc.sync.dma_start(out=gidx_dram[g], in_=gsm[0:9])
                nc.vector.tensor_scalar(y0g[0:9], wy1g[0:9], -1.0, 1.0,
                                        MULT, ADD)
                nc.vector.tensor_scalar(x0g[0:9], wx1g[0:9], -1.0, 1.0,
                                        MULT, ADD)
                wy0g, wx0g = y0g, x0g
                cols4 = slice(cb0, cb0 + GSTR)
                nc.vector.tensor_tensor(cT[r32, cols4], wy0g[0:9], wx0g[0:9],
                                        MULT)
                nc.vector.tensor_tensor(cB[r32, cols4], wy1g[0:9], wx0g[0:9],
                                        MULT)
                ctmp = gscr.tile([16, GSTR], BF16, tag="ctmp")
                btmp = gscr.tile([16, GSTR], BF16, tag="btmp")
                nc.vector.tensor_tensor(ctmp[0:9], wy0g[0:9], wx1g[0:9], MULT)
                nc.sync.dma_start(
                    out=cT[PBLK * b + 16:PBLK * b + 25, cols4], in_=ctmp[0:9])
                nc.vector.tensor_tensor(btmp[0:9], wy1g[0:9], wx1g[0:9], MULT)
                nc.sync.dma_start(
                    out=cB[PBLK * b + 16:PBLK * b + 25, cols4], in_=btmp[0:9])


            for r in range(8):
                nc.sync.dma_start(
                    out=gidx_w[r * 16:(r + 1) * 16, :, :, :],
                    in_=bass.AP(gidx_dram, 0,
                                [[1, 16], [16, NGROUP * KK * 64]]),
                )

            for g in range(NGROUP):
                b = g // 4
                cb0 = (g % 4) * GSTR
                # deformable conv for this group
                psy2 = ppy.tile([C_OUT, 2, CHUNK], F32, tag="psy2")
                for k in range(KK):
                    gt = wk3.tile([128, 2, GSTR], BF16, tag="gt")
                    if no_gather:
                        nc.vector.memset(gt[:], 0.0)
                    else:
                        nc.gpsimd.dma_gather(
                            out_ap=gt[:], in_ap=tok_dram[:],
                            idxs_ap=(cidx[:] if const_idx else gidx_w[:, g, k, :]),
                            num_idxs=GSTR, num_idxs_reg=GSTR, elem_size=256,
                            transpose=True, single_packet=False,
                        )
                    selk = sel2[:, b * 9 + k, :]
                    for cc in range(2):
                        cols = slice(cb0 + cc * CHUNK, cb0 + (cc + 1) * CHUNK)
                        ct2 = ps6.tile([128, 2, CHUNK], F32, tag="ct2")
                        nc.tensor.matmul(ct2[:, 0, :], selk, cT[:, cols],
                                         start=True, stop=True)
                        nc.tensor.matmul(ct2[:, 1, :], selk, cB[:, cols],
                                         start=True, stop=True)
                        ctb = work.tile([128, 2, CHUNK], BF16, tag="ctb")
                        nc.scalar.copy(ctb[:], ct2[:])
                        gm = work.tile([128, 2, CHUNK], BF16, tag="gm")
                        nc.vector.tensor_tensor(
                            gm[:], gt[:, :, cc * CHUNK:(cc + 1) * CHUNK],
                            ctb[:], MULT)
                        nc.tensor.matmul(psy2[:, cc, :], wdupS[:, k, :],
                                         gm[:, 0, :], start=(k == 0),
                                         stop=False, skip_group_check=True)
                        nc.tensor.matmul(psy2[:, cc, :], wdupS[:, k, :],
                                         gm[:, 1, :], start=False,
                                         stop=(k == KK - 1),
                                         skip_group_check=True)
                n0 = g * GSTR
                nc.scalar.add(
                    ydef[:, n0:n0 + GSTR],
                    psy2[:].rearrange("p a b -> p (a b)"), bd[:])
                bn_windows((g + 1) * GSTR)

            # ---- stage 7 tail: BN stats + AllReduce ----
            st2 = big.tile([C_OUT, 2], F32, tag="st2")
            nc.vector.tensor_reduce(
                st2[:, 0:1], ssum[:, 0:7], axis=mybir.AxisListType.X, op=ADD)
            nc.vector.tensor_reduce(
                st2[:, 1:2], ssq[:, 0:7], axis=mybir.AxisListType.X, op=ADD)
            nc.sync.dma_start(out=stats_in[:], in_=st2[:])
            if no_cc:
                nc.sync.dma_start(out=stats_out[:], in_=stats_in[:])
            else:
                nc.gpsimd.collective_compute(
                    "AllReduce", ADD, replica_groups=[list(range(n_cores))],
                    ins=[stats_in[:]], outs=[stats_out[:]])
            stg2 = big.tile([C_OUT, 2], F32, tag="stg2")
            nc.sync.dma_start(out=stg2[:], in_=stats_out[:])

            # ---- stage 8: BN affine + ReLU + store ----
            gam = big.tile([C_OUT, 1], F32, tag="gamt")
            bet = big.tile([C_OUT, 1], F32, tag="bett")
            nc.sync.dma_start(out=gam[:], in_=gam_c[:])
            nc.sync.dma_start(out=bet[:], in_=bet_c[:])
            NTOT = float(n_cores * H * W)
            mean = big.tile([C_OUT, 1], F32, tag="mean")
            var = big.tile([C_OUT, 1], F32, tag="var")
            nc.vector.tensor_scalar(mean[:], stg2[:, 0:1], 1.0 / NTOT, None, MULT)
            nc.vector.tensor_scalar(var[:], stg2[:, 1:2], 1.0 / NTOT, None, MULT)
            m2 = big.tile([C_OUT, 1], F32, tag="m2")
            nc.vector.tensor_tensor(m2[:], mean[:], mean[:], MULT)
            nc.vector.tensor_tensor(var[:], var[:], m2[:], SUB)
            nc.vector.tensor_scalar(var[:], var[:], EPS, None, ADD)
            sd = big.tile([C_OUT, 1], F32, tag="sd")
            nc.scalar.activation(sd[:], var[:], mybir.ActivationFunctionType.Sqrt)
            rsd = big.tile([C_OUT, 1], F32, tag="rsd")
            nc.vector.reciprocal(rsd[:], sd[:])
            aa = big.tile([C_OUT, 1], F32, tag="aa")
            bb2 = big.tile([C_OUT, 1], F32, tag="bb2")
            nc.vector.tensor_tensor(aa[:], gam[:], rsd[:], MULT)
            nc.vector.tensor_tensor(bb2[:], aa[:], mean[:], MULT)
            nc.vector.tensor_tensor(bb2[:], bet[:], bb2[:], SUB)
            for r in range(7):
                vap = yv[:, 1 + r * 16:1 + (r + 1) * 16, 1:113]
                ofin = s78.tile([C_OUT, 16 * W], F32, tag="ofin")
                nc.scalar.activation(
                    ofin[:].rearrange("p (a b) -> p a b", a=16), vap,
                    mybir.ActivationFunctionType.Relu, bias=bb2[:], scale=aa[:])
                nc.sync.dma_start(
                    out=y_out[:, r * 16:(r + 1) * 16, :],
                    in_=ofin[:].rearrange("p (a b) -> p a b", a=16))
            s78cm.__exit__(None, None, None)
            ppycm.__exit__(None, None, None)
            ps6cm.__exit__(None, None, None)
            ps3cm.__exit__(None, None, None)
            main_cm.__exit__(None, None, None)

            if loop is not None:
                loop.__exit__(None, None, None)

    nc.compile()
    return nc


def kernel(x, w_off, b_off, w_dcn, b_dcn, gamma, beta):
    from concourse.bass_utils import run_bass_kernel_spmd
    x = np.asarray(x, np.float32)
    nc = build_nc(
        np.asarray(w_off, np.float32), np.asarray(b_off, np.float32),
        np.asarray(w_dcn, np.float32), np.asarray(b_dcn, np.float32),
        np.asarray(gamma, np.float32), np.asarray(beta, np.float32),
    )
    in_maps = [{"x": np.ascontiguousarray(x[b])} for b in range(8)]
    res = run_bass_kernel_spmd(nc, in_maps, list(range(8)))
    return np.stack([res.results[b]["y"] for b in range(8)], 0).astype(np.float32)
